# revision 10
# baseline (speedup 1.0000x reference)
"""Trainium2 Bass kernel for nn_CCFLoss (masked-MSE heat/offset losses + argmax-gathered
class-balanced BCE), data-parallel over batch across 8 NeuronCores.

v5: the three masked-MSE sums are computed as Frobenius inner products
    sum((p-t)*w)^2 = <d^2, w^2> = trace((d^2)^T (w^2))
so the DVE only does subtractions (plus the exact f32 argmax scan), the ACT
engine does the elementwise squares, and the otherwise-idle TensorE contracts
everything into a single accumulating [128,128] PSUM bank whose diagonal the
host sums. GPSIMD is left idle on purpose: it shares an SBUF port with the
DVE, and any GPSIMD op measurably stretches concurrent DVE ops ~4x.

HBM traffic: 14 B/elem (ht f32 for the exact argmax tie-break, heat_pred+mask
as fp8e3 - they are only read by 1x-rate ops whose cost is dtype-independent -
offsets as bf16), vs 28 B/elem for the all-f32 baseline.

Layout per core (2 batches = 22 images): 5 groups of 4 images as [32, 2048]
blocks stacked on partitions -> [128, 2048] tiles, plus one tail group of 2
images as [64, 1024] -> [128, 1024]. Big groups amortize per-op overhead and
semaphores. hp|m and oyp|oxp, oyt|oxt are host-concatenated into single DMA
blocks. dh^2 runs on ACT for 4 of the 5 big groups and on DVE otherwise,
balancing the two engines at ~52 us each.

Host: sums diag / n_el for the MSE part, picks the global argmax per (b,c)
from per-partition top-1s, gathers clss_* at those 176 locations, finishes the
masked BCE means on scalars in float64.
"""
import sys

if "/opt/trn_rl_repo" not in sys.path:
    sys.path.insert(0, "/opt/trn_rl_repo")

import numpy as np
import ml_dtypes

B, C, H, W = 16, 11, 256, 256
P = 128
NCORES = 8
BPC = B // NCORES          # batches per core
NPAIR = BPC * C            # images per core (22)
# (n_images, img_partitions, img_freedim) per group; 5x4 + 1x2
GROUPS = [(4, 32, 2048)] * 5 + [(2, 64, 1024)]
NGA = 5                    # class-A group count
N_V_CHANNELS = 5

_STATE = {}


def _pos_weight(samples):
    s = np.asarray(samples, dtype=np.float64)
    beta = (s - 1.0) / s
    en = (1.0 - np.power(beta, s)) / (1.0 - beta)
    w = 1.0 / (en + 1e-5)
    return float(w[1] / (w[0] + 1e-5))


POS_W_V = _pos_weight([8000.0, 2000.0])
POS_W_D = _pos_weight([7000.0, 2000.0 + 1000.0])


def _build():
    import concourse.bacc as bacc
    import concourse.tile as tile
    import concourse.mybir as mybir

    f32 = mybir.dt.float32
    bf16 = mybir.dt.bfloat16
    fp8 = mybir.dt.float8e3
    u32 = mybir.dt.uint32
    SQUARE = mybir.ActivationFunctionType.Square

    nc = bacc.Bacc("TRN2", target_bir_lowering=False, debug=False)
    ins = {}
    for cls, ng, fd in (("A", NGA, 2048), ("B", 1, 1024)):
        ins["ht" + cls] = nc.dram_tensor("ht" + cls, [ng, P, fd], f32,
                                         kind="ExternalInput").ap()
        ins["hpm" + cls] = nc.dram_tensor("hpm" + cls, [ng, P, 2 * fd], fp8,
                                          kind="ExternalInput").ap()
        ins["oyxp" + cls] = nc.dram_tensor("oyxp" + cls, [ng, P, 2 * fd], bf16,
                                           kind="ExternalInput").ap()
        ins["oyxt" + cls] = nc.dram_tensor("oyxt" + cls, [ng, P, 2 * fd], bf16,
                                           kind="ExternalInput").ap()
    diag_d = nc.dram_tensor("diag", [P, P], f32, kind="ExternalOutput").ap()
    vals_d = nc.dram_tensor("vals8", [P, 8 * 6], f32, kind="ExternalOutput").ap()
    idx_d = nc.dram_tensor("idx8", [P, 8 * 6], u32, kind="ExternalOutput").ap()

    n_mm = sum(3 * (fd // 128) for _, _, fd in GROUPS)
    with tile.TileContext(nc) as tc:
        with tc.tile_pool(name="ins", bufs=3) as ipool, \
             tc.tile_pool(name="work", bufs=3) as wpool, \
             tc.tile_pool(name="acc", bufs=1) as apool, \
             tc.tile_pool(name="ps", bufs=1, space="PSUM") as pspool:
            vals_t = apool.tile([P, 8 * 6], f32)
            idx_t = apool.tile([P, 8 * 6], u32)
            psum_t = pspool.tile([P, P], f32)

            mm_i = 0
            for g, (nimg, pi, fd) in enumerate(GROUPS):
                cls = "A" if g < NGA else "B"
                gi = g if g < NGA else 0
                # tiles are allocated at class-A width; class B uses a prefix
                t = {}
                for j, (name, dt, w, wa) in enumerate((
                        ("ht", f32, fd, 2048), ("hpm", fp8, 2 * fd, 4096),
                        ("oyxp", bf16, 2 * fd, 4096), ("oyxt", bf16, 2 * fd, 4096))):
                    full = ipool.tile([P, wa], dt, tag=name)
                    tt = full[:, :w]
                    # first group: co-issue from ACT so transfers start on
                    # whichever sequencer boots first
                    eng = nc.scalar if (g == 0 and j % 2 == 1) else nc.sync
                    eng.dma_start(out=tt, in_=ins[name + cls][gi])
                    t[name] = tt

                # per-partition top-8 of ht (f32, exact) - covers all images
                v8 = vals_t[:, 8 * g:8 * g + 8]
                nc.vector.max(out=v8, in_=t["ht"])
                nc.vector.max_index(out=idx_t[:, 8 * g:8 * g + 8],
                                    in_max=v8, in_values=t["ht"])

                # squares of the weights (ACT, 1x rate, any input dtype)
                ht2_t = wpool.tile([P, 2048], bf16, tag="ht2")
                ht2 = ht2_t[:, :fd]
                nc.scalar.activation(ht2, t["ht"], SQUARE)
                m2_t = wpool.tile([P, 2048], bf16, tag="m2")
                m2 = m2_t[:, :fd]
                nc.scalar.activation(m2, t["hpm"][:, fd:], SQUARE)

                # diffs (DVE) and their squares (DVE/ACT balanced)
                dh_t = wpool.tile([P, 2048], bf16, tag="dh")
                dh = dh_t[:, :fd]
                nc.vector.tensor_sub(out=dh, in0=t["hpm"][:, :fd],
                                     in1=t["ht"])
                dh2_t = wpool.tile([P, 2048], bf16, tag="dh2")
                dh2 = dh2_t[:, :fd]
                if g == 0 or cls == "B":
                    nc.vector.tensor_mul(out=dh2, in0=dh, in1=dh)
                else:
                    nc.scalar.activation(dh2, dh, SQUARE)
                dyx_t = wpool.tile([P, 4096], bf16, tag="dyx")
                dyx = dyx_t[:, :2 * fd]
                nc.vector.tensor_sub(out=dyx, in0=t["oyxp"],
                                     in1=t["oyxt"])
                dyx2_t = wpool.tile([P, 4096], bf16, tag="dyx2")
                dyx2 = dyx2_t[:, :2 * fd]
                nc.scalar.activation(dyx2, dyx, SQUARE)

                # accumulate sum(d^2 * w^2) = trace((d^2)^T (w^2)) chunkwise
                # into one PSUM bank; host reads the diagonal. lhsT=weight^2 so
                # the off-term reuses each ht^2 chunk for both dy and dx.
                nch = fd // 128
                for c in range(nch):
                    s = slice(128 * c, 128 * c + 128)
                    nc.tensor.matmul(psum_t[:], lhsT=m2[:, s], rhs=dh2[:, s],
                                     start=(mm_i == 0), stop=(mm_i == n_mm - 1))
                    mm_i += 1
                    nc.tensor.matmul(psum_t[:], lhsT=ht2[:, s], rhs=dyx2[:, s],
                                     start=False, stop=(mm_i == n_mm - 1))
                    mm_i += 1
                    s2 = slice(fd + 128 * c, fd + 128 * c + 128)
                    nc.tensor.matmul(psum_t[:], lhsT=ht2[:, s], rhs=dyx2[:, s2],
                                     start=False, stop=(mm_i == n_mm - 1))
                    mm_i += 1

            diag_s = apool.tile([P, P], f32)
            nc.scalar.copy(out=diag_s[:], in_=psum_t[:])
            nc.sync.dma_start(out=diag_d, in_=diag_s[:])
            nc.sync.dma_start(out=vals_d, in_=vals_t[:])
            nc.sync.dma_start(out=idx_d, in_=idx_t[:])

    nc.compile()
    return nc


def _get_nc():
    if "nc" not in _STATE:
        _STATE["nc"] = _build()
    return _STATE["nc"]


def _softplus(x):
    return np.log1p(np.exp(-np.abs(x))) + np.maximum(x, 0.0)


def run_device(in_maps, **kwargs):
    from concourse.bass_utils import run_bass_kernel_spmd
    nc = _get_nc()
    return run_bass_kernel_spmd(nc, in_maps, core_ids=list(range(NCORES)), **kwargs)


def make_in_maps(inp):
    fp8 = ml_dtypes.float8_e3m4
    bf16 = ml_dtypes.bfloat16
    src = {"ht": np.ascontiguousarray(inp["heat_targets"], dtype=np.float32),
           "hp": np.ascontiguousarray(inp["heat_predictions"], dtype=np.float32),
           "m": np.ascontiguousarray(inp["masks"], dtype=np.float32),
           "oyp": np.ascontiguousarray(inp["offy_predictions"], dtype=np.float32),
           "oxp": np.ascontiguousarray(inp["offx_predictions"], dtype=np.float32),
           "oyt": np.ascontiguousarray(inp["offy_targets"], dtype=np.float32),
           "oxt": np.ascontiguousarray(inp["offx_targets"], dtype=np.float32)}
    # per tensor: class-A [NCORES, 5, 128, 2048] and class-B [NCORES, 128, 1024]
    parts = {}
    for name, a in src.items():
        a = a.reshape(NCORES, NPAIR, H * W)
        parts[name] = (a[:, :20].reshape(NCORES, NGA, P, 2048),
                       a[:, 20:].reshape(NCORES, P, 1024))
    in_maps = []
    for k in range(NCORES):
        im = {}
        for cls, i in (("A", 0), ("B", 1)):
            im["ht" + cls] = np.ascontiguousarray(parts["ht"][i][k])
            im["hpm" + cls] = np.ascontiguousarray(np.concatenate(
                [parts["hp"][i][k].astype(fp8), parts["m"][i][k].astype(fp8)],
                axis=-1))
            im["oyxp" + cls] = np.ascontiguousarray(np.concatenate(
                [parts["oyp"][i][k].astype(bf16), parts["oxp"][i][k].astype(bf16)],
                axis=-1))
            im["oyxt" + cls] = np.ascontiguousarray(np.concatenate(
                [parts["oyt"][i][k].astype(bf16), parts["oxt"][i][k].astype(bf16)],
                axis=-1))
        in_maps.append(im)
    return in_maps


def finish_host(results, inp):
    """Combine per-core device outputs into the final scalar loss (float64 host math)."""
    cp = np.asarray(inp["clss_predictions"], dtype=np.float32).reshape(B, C, H * W)
    ct = np.asarray(inp["clss_targets"], dtype=np.float32).reshape(B, C, H * W)
    v_w = float(np.asarray(inp["v_loss_weight"]))
    d_w = float(np.asarray(inp["d_loss_weight"]))

    mse_sum = 0.0
    g_pred = np.zeros((B, C), dtype=np.float64)
    g_tgt = np.zeros((B, C), dtype=np.float64)
    for k in range(NCORES):
        out = results[k]
        mse_sum += float(np.trace(np.asarray(out["diag"], dtype=np.float64)))
        pm = np.asarray(out["vals8"]).reshape(P, 6, 8)[:, :, 0]
        ji = np.asarray(out["idx8"]).reshape(P, 6, 8)[:, :, 0]
        i = 0
        for g, (nimg, pi, fd) in enumerate(GROUPS):
            for h in range(nimg):
                b = k * BPC + i // C
                c = i % C
                rows = slice(pi * h, pi * h + pi)
                p_star = int(np.argmax(pm[rows, g]))  # first max part == lowest flat
                flat = p_star * fd + int(ji[pi * h + p_star, g])
                g_pred[b, c] = cp[b, c, flat]
                g_tgt[b, c] = ct[b, c, flat]
                i += 1

    n_el = float(B * C * H * W)
    mse_loss = mse_sum / n_el   # heat + offy + offx (all weights are 1.0)

    valid = g_tgt >= 0.0
    is_v = (np.arange(C) < N_V_CHANNELS)[None, :]
    v_mask = (valid & is_v).astype(np.float64)
    d_mask = (valid & ~is_v).astype(np.float64)

    x = g_pred
    sp_neg = _softplus(-x)
    sp_pos = _softplus(x)

    l_v = POS_W_V * g_tgt * sp_neg + (1.0 - g_tgt) * sp_pos
    v_cls = (l_v * v_mask).sum() / max(v_mask.sum(), 1.0)
    y_d = (g_tgt >= 1.0).astype(np.float64)
    l_d = POS_W_D * y_d * sp_neg + (1.0 - y_d) * sp_pos
    d_cls = (l_d * d_mask).sum() / max(d_mask.sum(), 1.0)

    loss = mse_loss + v_cls * v_w + d_cls * d_w
    return np.float32(loss)


def kernel(**inputs):
    inp = {k: np.asarray(v) for k, v in inputs.items()}
    in_maps = make_in_maps(inp)
    res = run_device(in_maps)
    return finish_host(res.results, inp)
